# revision 29
# baseline (speedup 1.0000x reference)
"""Trainium2 Bass kernel for nn_Entropy (histogram_binning): per-pixel Shannon
entropy of a 5x5-window KDE histogram over 256 intensity bins.

v2 design (Exp-based front end, single activation table, custom DVE ops):
  k(x,b) = sig'(10(x-b)) = u/(1+u)^2 with u = exp(-10|x-b|).
  Layout per (image): h on partitions, free = (bin-block, w-inner) with
  4 zero-pads per 100-col block (plus 5 lead / 2 trail pads per half-chunk).
  Pipeline per superchunk (8 bins x 96 w = 768 cols, 16 sc per half-pair):
    TensorE  D = x - b          (K=97 matmul: stationary [x^T; 1], moving
                                 delta-selector + (-b) row, fp32)
    DVE      a = |D - 128h|     (custom ABSDEV, fp16, strided into padded
                                 chunk; pads preset to 1e4 so exp(pad) = 0)
    ScalarE  u = Exp(-10 a)     (fp16)
    DVE      v = u + u^2(c2+c3u) (custom VPOLY ~ u/(1+u), rel err ~0.5%)
    DVE      A5 = running 5-window of k = v(1-v)  (custom WIN5K scan op:
             state += k(v[i]) - k(v[i-5]); block pads make all SAME edges
             exact with zero fixups)
    TensorE  P = band @ A5      (fp16 H-window band matmul -> PSUM = q)
    ScalarE  L = Ln(q + 2e-6)   (batched; eps-shift identity removes the
                                 per-pixel 1/S scale from the log argument)
    DVE      e = q * L          (PSUM x fp16 -> e-stripe fp16)
  Per half: 7-level pairwise tree-reduce of e over bins -> T = sum_b q ln q.
  Analytic S-path (5 taps of the same fp16 Exp/poly chain on [96,288]) gives
  S = sum_b q; E = ln(S+EPS) - T/(S+EPS).  Exp/Ln/Identity/Abs share one
  activation table -> no ACT table reloads.
  Sharding: B*C = 24 images, 3 per core across 8 cores; no collectives.
"""

import sys

sys.path.insert(0, "/opt/trn_rl_repo")

import numpy as np

H = 96
W = 96
NBH = 128          # bins per half
NIMG = 3
NCORES = 8
EPS = 1e-10
EPS1 = 2e-6
C2P, C3P = -0.89877895, 0.43582129   # v-poly coeffs: v = u + u^2(C2P + C3P u)
BLK = 100          # per-bin block: 4 pads + 96 w
NSC = 16           # superchunks per image (8 bins each, both halves)
BPS = 8            # bins per superchunk per half
HCH = 5 + BPS * BLK + 2   # half-chunk cols: 5 lead + 800 + 2 trail = 807
PAD = 1e4          # a-pad value: exp(-10*PAD) == 0

_CACHE = {}


def _register_dve_ops():
    import concourse.dve_ops as dve_ops
    from concourse.dve_ops import DveOp
    from concourse.dve_spec import (
        C0, C1, AluOp, One, Spec, Src0, Src1, maxx, scan, sq,
    )

    def register(op):
        if op.name not in dve_ops._SUB_OPCODE_FOR_NAME:
            dve_ops.OPS.append(op)
            dve_ops._SUB_OPCODE_FOR_NAME[op.name] = (
                dve_ops._CUSTOM_DVE_ROW_BASE + len(dve_ops.OPS) - 1
            )
        else:
            op = next(o for o in dve_ops.OPS if o.name == op.name)
        return op

    absdev = register(DveOp(
        "ABSDEV_ANT",
        Spec(body=maxx(Src0 - C0, C0 - Src0),
             reference=lambda in0, in1, c0, c1, c2: np.abs(
                 in0.astype(np.float32) - c0).astype(np.float32)),
        subdim=False,
        uops_sha={"v3": "a5866c869c7d6e30", "v4": "006fe4b232e6035a"}))

    vpoly = register(DveOp(
        "VPOLY_ANT",
        Spec(body=Src0 + sq(Src0) * (C0 + C1 * Src0),
             reference=lambda in0, in1, c0, c1, c2: (
                 lambda u: (u + u * u * (c0 + c1 * u)).astype(np.float32)
             )(in0.astype(np.float32))),
        subdim=False,
        perf_en={"v3": True, "v4": True},
        uops_sha={"v3": "217961e937d92645", "v4": "56741f276e7f1259"}))

    win5 = register(DveOp(
        "WIN5K_ANT",
        Spec(body=scan(AluOp.ADD, Src0 * (One - Src0) - Src1 * (One - Src1)),
             reference=lambda in0, in1, c0, c1, c2: np.cumsum(
                 in0.astype(np.float32) * (1 - in0.astype(np.float32))
                 - in1.astype(np.float32) * (1 - in1.astype(np.float32)),
                 axis=-1, dtype=np.float32)),
        subdim=False,
        perf_en={"v3": True, "v4": True},
        uops_sha={"v3": "9d91f28b1ae18abb", "v4": "1425a9f273284709"}))

    return absdev, vpoly, win5


def _patch_act_tables():
    """Force Exp and Ln onto one shared activation table so the table-load
    pass never ping-pongs between per-function tables inside the main loop."""
    import concourse.hw_specs as hw_specs
    from concourse import bacc, mybir

    if getattr(hw_specs, "_ant_act_patch", False):
        return
    AF = mybir.ActivationFunctionType
    orig = hw_specs.get_activation_tables

    def patched(arch):
        tabs = orig(arch)
        out = {}
        for name, s in tabs.items():
            if name == "natural_log_exp_and_others":
                out[name] = set(s)
            else:
                out[name] = set(s) - {AF.Exp, AF.Ln}
        return out

    hw_specs.get_activation_tables = patched
    bacc.get_activation_tables = patched
    hw_specs._ant_act_patch = True


def _build_consts():
    # selector moving operand [97, 128*96]: col c = b_local*96 + w.
    # rows k<96: delta(k == w); row 96: -b_local.
    sel = np.zeros((97, NBH * W), dtype=np.float16)
    cols = np.arange(NBH * W)
    bl = cols // W
    w = cols % W
    sel[w, cols] = 1.0
    sel[96, :] = -bl.astype(np.float16)
    hh = np.arange(H)
    band = (np.abs(hh[:, None] - hh[None, :]) <= 2).astype(np.float16)
    return sel, band


def _emit_kernel(nc, tc, ctx, ins, outs, ops):
    from concourse import mybir

    f32 = mybir.dt.float32
    f16 = mybir.dt.float16
    i32 = mybir.dt.int32
    AF = mybir.ActivationFunctionType
    OP = mybir.AluOpType

    ABSDEV, VPOLY, WIN5 = ops
    x_d, xt_d, sel_d, band_d = ins
    (ent_d,) = outs
    NW = NIMG * W

    consts = ctx.enter_context(tc.tile_pool(name="consts", bufs=1))
    sm = ctx.enter_context(tc.tile_pool(name="sm", bufs=1))
    apool = ctx.enter_context(tc.tile_pool(name="ap", bufs=2))
    upool = ctx.enter_context(tc.tile_pool(name="up", bufs=2))
    vpool = ctx.enter_context(tc.tile_pool(name="vp", bufs=2))
    a5pool = ctx.enter_context(tc.tile_pool(name="a5p", bufs=2))
    lpool = ctx.enter_context(tc.tile_pool(name="lp", bufs=2))
    epool = ctx.enter_context(tc.tile_pool(name="ep", bufs=2))
    tpool = ctx.enter_context(tc.tile_pool(name="tp", bufs=1))
    dpsum = ctx.enter_context(tc.tile_pool(name="dps", bufs=2, space="PSUM"))
    ppsum = ctx.enter_context(tc.tile_pool(name="pps", bufs=1, space="PSUM"))

    # ---- constants / inputs ----
    band_sb = consts.tile([H, H], f16)
    nc.sync.dma_start(band_sb[:], band_d[:])

    xall = consts.tile([H, NW], f32)
    xt32 = consts.tile([H, NIMG * H], f32)
    for i in range(NIMG):
        nc.sync.dma_start(xall[:, i * W:(i + 1) * W], x_d[i])
        nc.sync.dma_start(xt32[:, i * H:(i + 1) * H], xt_d[i])
    # stationaries: st_a = [round(x)^T ; ones], st_b = [frac^T ; zeros] (fp16)
    st_a = consts.tile([97, NIMG * H], f16)
    st_b = consts.tile([97, NIMG * H], f16)
    xti = consts.tile([H, NIMG * H], i32)
    nc.vector.tensor_copy(xti[:], xt32[:])
    xtif = consts.tile([H, NIMG * H], f32)
    nc.vector.tensor_copy(xtif[:], xti[:])
    nc.vector.tensor_copy(st_a[0:96, :], xtif[:])
    nc.vector.tensor_tensor(st_b[0:96, :], xt32[:], xtif[:], op=OP.subtract)
    nc.vector.memset(st_a[96:97, :], 1.0)
    nc.vector.memset(st_b[96:97, :], 0.0)

    # selector DMA'd in 16 chunks, spread over 4 queues, so the first
    # D-matmul starts early and the transfers run in parallel
    sel_sb = consts.tile([97, NBH * W], f16)
    for j in range(NSC):
        nc.gpsimd.dma_start(
            sel_sb[:, 768 * j:768 * (j + 1)], sel_d[:, 768 * j:768 * (j + 1)])

    bias_tiles = {}

    def bias_ap(val):
        if val not in bias_tiles:
            t = consts.tile([H, 1], f32, tag=f"bias{val}")
            nc.vector.memset(t[:], val)
            bias_tiles[val] = t
        return bias_tiles[val][:]

    # =====================  S path ([96, 288])  =====================
    ni = sm.tile([H, NW], i32)
    nc.vector.tensor_copy(ni[:], xall[:])
    nf = sm.tile([H, NW], f32)
    nc.vector.tensor_copy(nf[:], ni[:])
    ufrac = sm.tile([H, NW], f32)
    nc.vector.tensor_tensor(ufrac[:], xall[:], nf[:], op=OP.subtract)
    taps = (-2, -1, 0, 1, 2)
    atap = sm.tile([H, 5, NW], f16)
    for oi, o in enumerate(taps):
        # a_o = |ufrac - o|; ufrac in [-0.5, 0.5] (i32 copy rounds to nearest)
        nc.vector._custom_dve(
            ABSDEV, out=atap[:, oi, :], in0=ufrac[:], s0=float(o))
    utap = sm.tile([H, 5, NW], f16)
    nc.scalar.activation(utap[:], atap[:], AF.Exp, scale=-10.0)
    vtap = sm.tile([H, 5, NW], f16)
    nc.vector._custom_dve(VPOLY, out=vtap[:], in0=utap[:], s0=C2P, s1=C3P)
    vsq = sm.tile([H, 5, NW], f16)
    nc.vector.tensor_tensor(vsq[:], vtap[:], vtap[:], op=OP.mult)
    ktap = sm.tile([H, 5, NW], f16)
    nc.vector.tensor_tensor(ktap[:], vtap[:], vsq[:], op=OP.subtract)

    spix = sm.tile([H, NW], f32)
    nc.vector.tensor_copy(spix[:], ktap[:, 2, :])  # o=0 tap, always valid
    for oi, o in enumerate(taps):
        if o == 0:
            continue
        m = sm.tile([H, NW], f32, tag=f"m{o}")
        if o < 0:
            nc.vector.tensor_scalar(m[:], nf[:], float(-o), None, op0=OP.is_ge)
        else:
            nc.vector.tensor_scalar(
                m[:], nf[:], float(255 - o), None, op0=OP.is_le)
        tm = sm.tile([H, NW], f32, tag=f"tm{o}")
        nc.vector.tensor_tensor(tm[:], m[:], ktap[:, oi, :], op=OP.mult)
        nc.vector.tensor_tensor(spix[:], spix[:], tm[:], op=OP.add)
    spix16 = sm.tile([H, NW], f16)
    nc.vector.tensor_copy(spix16[:], spix[:])

    ps_s = ppsum.tile([H, 2048], f32, tag="pp")
    nc.tensor.matmul(ps_s[:, 0:NW], band_sb[:], spix16[:], start=True, stop=True)
    sh = sm.tile([H, NW], f32)
    nc.scalar.copy(sh[:], ps_s[:, 0:NW])
    shp = sm.tile([H, NIMG, W + 4], f32)
    nc.vector.memset(shp[:], 0.0)
    for i in range(NIMG):
        nc.vector.tensor_copy(shp[:, i, 2:2 + W], sh[:, i * W:(i + 1) * W])
    swin = sm.tile([H, NIMG, W], f32)
    nc.vector.tensor_tensor(swin[:], shp[:, :, 0:W], shp[:, :, 1:1 + W], op=OP.add)
    for j in (2, 3, 4):
        nc.vector.tensor_tensor(swin[:], swin[:], shp[:, :, j:j + W], op=OP.add)
    sw_flat = swin[:].rearrange("p a b -> p (a b)")
    rtile = sm.tile([H, NW], f32)
    nc.vector.tensor_scalar(rtile[:], sw_flat, EPS, None, op0=OP.add)
    nc.vector.reciprocal(rtile[:], rtile[:])
    lns = sm.tile([H, NW], f32)
    nc.scalar.activation(lns[:], sw_flat, AF.Ln, bias=bias_ap(EPS))

    # =====================  main path  =====================
    QL = sm.tile([H, NW], f32)
    tacc2 = sm.tile([H, 2, W], f32)

    npads_set = [0]

    for i in range(NIMG):
        nc.vector.memset(tacc2[:], 0.0)
        for sc in range(NSC):
            dt = dpsum.tile([H, 768], f32, tag="d")
            mvbase = 768 * sc
            for lo, hi in ((0, 512), (512, 768)):
                nc.tensor.matmul(
                    dt[:, lo:hi], st_a[:, i * H:(i + 1) * H],
                    sel_sb[:, mvbase + lo:mvbase + hi], start=True, stop=False)
                nc.tensor.matmul(
                    dt[:, lo:hi], st_b[:, i * H:(i + 1) * H],
                    sel_sb[:, mvbase + lo:mvbase + hi], start=False, stop=True)

            at = apool.tile([H, 2 * HCH], f16, tag="a")
            if npads_set[0] < 2:
                nc.vector.memset(at[:], PAD)
                npads_set[0] += 1
            dst0 = at[:, 5:5 + BPS * BLK] \
                .rearrange("p (b z) -> p b z", z=BLK)[:, :, 4:BLK]
            nc.vector._custom_dve(ABSDEV, out=dst0, in0=dt[:, 0:768], s0=0.0)
            dst1 = at[:, HCH + 5:HCH + 5 + BPS * BLK] \
                .rearrange("p (b z) -> p b z", z=BLK)[:, :, 4:BLK]
            nc.scalar.activation(dst1, dt[:, 0:768], AF.Abs, bias=bias_ap(-128.0))
            ut = upool.tile([H, 2 * HCH], f16, tag="u")
            nc.scalar.activation(ut[:], at[:], AF.Exp, scale=-10.0)
            vt = vpool.tile([H, 2 * HCH], f16, tag="v")
            nc.vector._custom_dve(VPOLY, out=vt[:], in0=ut[:], s0=C2P, s1=C3P)
            a5 = a5pool.tile([H, 2 * HCH], f16, tag="a5")
            nc.vector._custom_dve(
                WIN5, out=a5[:, 0:2 * HCH - 5], in0=vt[:, 5:2 * HCH],
                in1=vt[:, 0:2 * HCH - 5])

            pt = ppsum.tile([H, 2048], f32, tag="pp")
            for h in range(2):
                for pp in range(2):
                    off = h * HCH + 400 * pp + 6
                    mvap = a5[:, off:off + 400] \
                        .rearrange("p (b z) -> p b z", z=BLK)[:, :, 0:96]
                    nc.tensor.matmul(
                        pt[:, 1024 * h + 512 * pp:1024 * h + 512 * pp + 384],
                        band_sb[:], mvap, start=True, stop=True)
            lt = lpool.tile([H, 1536], f16, tag="l")
            nc.scalar.activation(
                lt[:].rearrange("p (a b) -> p a b", b=384),
                pt[:].rearrange("p (a b) -> p a b", b=512)[:, :, 0:384],
                AF.Ln, bias=bias_ap(EPS1))
            et = epool.tile([H, 2 * 1152], f16, tag="e")
            # h0: e = q*L straight from PSUM on V
            nc.vector.tensor_tensor(
                et[:, 0:768].rearrange("p (a b) -> p a b", b=384),
                pt[:, 0:1024].rearrange("p (a b) -> p a b", b=512)[:, :, 0:384],
                lt[:, 0:768].rearrange("p (a b) -> p a b", b=384),
                op=OP.mult)
            # h1: evacuate q on Scalar, multiply on Pool
            qe = epool.tile([H, 768], f16, tag="qe")
            nc.scalar.copy(
                qe[:].rearrange("p (a b) -> p a b", b=384),
                pt[:, 1024:2048].rearrange("p (a b) -> p a b", b=512)[:, :, 0:384])
            nc.gpsimd.tensor_tensor(
                et[:, 1152:1920], qe[:], lt[:, 768:1536], op=OP.mult)
            # per-sc tree over the 8 bins, both halves per instruction
            e2 = et[:].rearrange("p (a b) -> p a b", b=1152)
            nc.gpsimd.tensor_tensor(
                e2[:, :, 768:1152], e2[:, :, 0:384], e2[:, :, 384:768], op=OP.add)
            nc.gpsimd.tensor_tensor(
                e2[:, :, 0:192], e2[:, :, 768:960], e2[:, :, 960:1152], op=OP.add)
            nc.gpsimd.tensor_tensor(
                e2[:, :, 192:288], e2[:, :, 0:96], e2[:, :, 96:192], op=OP.add)
            nc.gpsimd.tensor_tensor(
                tacc2[:], tacc2[:], e2[:, :, 192:288], op=OP.add)

        nc.vector.tensor_tensor(
            QL[:, i * W:(i + 1) * W], tacc2[:, 0, :], tacc2[:, 1, :], op=OP.add)

    # E = lnS - r*T
    ent = sm.tile([H, NW], f32)
    nc.vector.tensor_tensor(ent[:], rtile[:], QL[:], op=OP.mult)
    nc.vector.tensor_tensor(ent[:], lns[:], ent[:], op=OP.subtract)
    for i in range(NIMG):
        nc.sync.dma_start(ent_d[i], ent[:, i * W:(i + 1) * W])


def _get_compiled():
    if "nc" in _CACHE:
        return _CACHE["nc"]
    from contextlib import ExitStack

    import concourse.tile as tile
    from concourse import bacc, mybir

    ops = _register_dve_ops()
    _patch_act_tables()

    f32 = mybir.dt.float32
    f16 = mybir.dt.float16
    nc = bacc.Bacc("TRN2", target_bir_lowering=False, debug=False)
    x_d = nc.dram_tensor("x_sh", [NIMG, H, W], f32, kind="ExternalInput").ap()
    xt_d = nc.dram_tensor("xt_sh", [NIMG, W, H], f32, kind="ExternalInput").ap()
    sel_d = nc.dram_tensor("sel", [97, NBH * W], f16, kind="ExternalInput").ap()
    band_d = nc.dram_tensor("band16", [H, H], f16, kind="ExternalInput").ap()
    ent_d = nc.dram_tensor("ent", [NIMG, H, W], f32, kind="ExternalOutput").ap()

    with tile.TileContext(nc) as tc:
        with ExitStack() as ctx:
            _emit_kernel(
                nc, tc, ctx, (x_d, xt_d, sel_d, band_d), (ent_d,), ops
            )
    nc.compile()
    _CACHE["nc"] = nc
    return nc


def make_in_maps(x):
    """x: full [8, 3, 96, 96] -> list of 8 per-core input dicts."""
    x = np.ascontiguousarray(np.asarray(x, dtype=np.float32))
    imgs = x.reshape(NCORES * NIMG, H, W)
    sel, band = _build_consts()
    in_maps = []
    for c in range(NCORES):
        sh = np.ascontiguousarray(imgs[c * NIMG:(c + 1) * NIMG])
        in_maps.append(
            {
                "x_sh": sh,
                "xt_sh": np.ascontiguousarray(sh.transpose(0, 2, 1)),
                "sel": sel,
                "band16": band,
            }
        )
    return in_maps


def kernel(x):
    """Full inputs in, full outputs out. x: [8, 3, 96, 96] f32."""
    from concourse.bass_utils import run_bass_kernel_spmd

    nc = _get_compiled()
    in_maps = make_in_maps(x)
    res = run_bass_kernel_spmd(nc, in_maps, list(range(NCORES)))
    out = np.stack([res.results[c]["ent"] for c in range(NCORES)])
    return out.reshape(8, 3, H, W).astype(np.float32)
